# revision 11
# baseline (speedup 1.0000x reference)
"""MoE head (gate + top-2 expert MLPs + load-balance aux) on 8 Trainium2 cores.

Strategy: expert-parallel. The gate (0.05% of FLOPs) is computed on host with
jax ops mirroring the reference exactly (so top-2 selection bit-matches the
reference); tokens are dispatched to core e = expert e. Each core runs two
GEMMs on its ~4.1k routed tokens:

  phase 1:  hT[H, n] = w1[e].T @ x_gathered.T   (f32r matmuls, f32 PSUM)
            fused epilogue: gelu(psum + b1) -> fp16 hT in DRAM
  phase 2:  y[n, C]  = hT.T @ w2[e]             (fp16 matmuls, f32 PSUM)
            fused epilogue: (psum + b2) * combine_weight -> f32

Host then scatter-adds each expert's rows back into the full [B, 1000] output.
Rows the reference zero-weights contribute exactly 0 there, so routed compute
is mathematically identical to the dense reference loop.
"""

import math
import os
import sys

import numpy as np

for _p in ("/opt/trn_rl_repo", "/root/.axon_site/_ro/trn_rl_repo"):
    if os.path.isdir(_p) and _p not in sys.path:
        sys.path.insert(0, _p)

D_MODEL = 1024
NUM_CLASSES = 1000
NUM_EXPERTS = 8
TOP_K = 2
HIDDEN = D_MODEL * 4
TEMPERATURE = 1.0

# padded class dim (multiple of 512 keeps every tile full)
C_PAD = 1024

_PROGRAM_CACHE = {}
_LAST_RUN = {}


def _gate(x, gate_w, gate_b):
    """Gate math mirroring reference() exactly (same jax ops, same platform
    the harness runs the reference on) so that top-2 selection and aux agree
    to the bit whenever possible. Falls back to numpy if jax is unavailable."""
    try:
        import jax
        import jax.numpy as jnp

        xj = jnp.asarray(x)
        logits = (xj @ jnp.asarray(gate_w) + jnp.asarray(gate_b)) / TEMPERATURE
        probs = jax.nn.softmax(logits, axis=-1)
        topk_vals, topk_idx = jax.lax.top_k(probs, TOP_K)
        combine_w = topk_vals / jnp.sum(topk_vals, axis=-1, keepdims=True)
        onehot = jax.nn.one_hot(topk_idx, NUM_EXPERTS, dtype=xj.dtype)
        frac = jnp.mean(jnp.sum(onehot, axis=1), axis=0) / TOP_K
        imp = jnp.mean(probs, axis=0)
        aux = NUM_EXPERTS * jnp.sum(frac * imp)
        return (
            np.asarray(topk_idx),
            np.asarray(combine_w),
            np.asarray(aux, dtype=np.float32),
        )
    except Exception:
        logits = (x @ gate_w + gate_b) / TEMPERATURE
        m = logits.max(-1, keepdims=True)
        p = np.exp(logits - m, dtype=np.float32)
        probs = p / p.sum(-1, keepdims=True)
        topk_idx = np.argsort(-probs, axis=-1, kind="stable")[:, :TOP_K]
        topk_vals = np.take_along_axis(probs, topk_idx, axis=-1)
        combine_w = topk_vals / topk_vals.sum(-1, keepdims=True)
        counts = np.bincount(topk_idx.ravel(), minlength=NUM_EXPERTS)
        frac = counts.astype(np.float32) / (x.shape[0] * TOP_K)
        imp = probs.mean(0)
        aux = np.float32(NUM_EXPERTS * np.sum(frac * imp))
        return topk_idx, combine_w, aux


def _build_program(n1, n2=None, d_model=D_MODEL, hidden=HIDDEN, c_pad=C_PAD,
                   repeat=1):
    """One SPMD Bass program: per-core routed-expert MLP.

    n1: padded token count for GEMM1 (multiple of 512 — the cached-kxn path
        needs full 512-wide N tiles).
    n2: token count for GEMM2/output (multiple of 128, n2 <= n1) — GEMM2 only
        reads the first n2 columns of hT, so the 512-rounding waste of GEMM1
        is not paid again in GEMM2.
    repeat > 1 re-runs the whole compute that many times (same tensors) —
    used only for differential wall-clock timing, never for results."""
    if n2 is None:
        n2 = n1
    import concourse.mybir as mybir
    import concourse.tile as tile
    from concourse import bacc
    from concourse.bass import ds
    from concourse.kernels.tile_matmul import (
        ShapeInfo,
        composable_matmul_tile_kernel,
        dma_from_dram_kxm,
        dma_to_dram_mxn,
    )

    P = 128
    KO1 = d_model // P      # k-outer chunks for GEMM1
    HO = hidden // P        # h chunks of 128
    MO = n2 // P            # output token chunks of 128

    nc = bacc.Bacc("TRN2", target_bir_lowering=False, debug=False, num_devices=8)

    f32 = mybir.dt.float32
    f32r = mybir.dt.float32r
    f16 = mybir.dt.float16

    xT_d = nc.dram_tensor("xT", [P, KO1, n1], f32r, kind="ExternalInput")
    w1_d = nc.dram_tensor("w1e", [P, KO1, hidden], f32r, kind="ExternalInput")
    b1_d = nc.dram_tensor("b1e", [P, HO], f32, kind="ExternalInput")
    w2_d = nc.dram_tensor("w2e", [P, HO, c_pad], f16, kind="ExternalInput")
    b2_d = nc.dram_tensor("b2e", [P, c_pad], f32, kind="ExternalInput")
    sc_d = nc.dram_tensor("scale", [P, MO], f32, kind="ExternalInput")
    y_d = nc.dram_tensor("y", [P, MO, c_pad], f32, kind="ExternalOutput")

    HT_SPLIT = max(1, hidden // 512)  # one hT shard per GEMM1 m-tile
    with tile.TileContext(nc) as tc:
        with tc.tile_pool(name="dram", bufs=1, space="DRAM") as dramp:
            hT_tiles = [
                dramp.tile([P, HO // HT_SPLIT, n1], f16, name=f"hT{j}")
                for j in range(HT_SPLIT)
            ]

            def _emit_once():
                # ---------------- phase 1: hT = gelu(w1.T @ xT + b1) ----------------
                with tc.tile_pool(name="xsb", bufs=1) as xp, \
                     tc.tile_pool(name="c1", bufs=1) as c1p, \
                     tc.tile_pool(name="kxm1", bufs=d_model // P + 1) as kxm1p:
                    NB1 = n1 // 512
                    xT_tiles = {}
                    for k in range(KO1):
                        for b in range(NB1):
                            t = xp.tile([P, 1, 512], f32r, name=f"xT{k}_{b}")
                            nc.sync.dma_start(
                                t[:, 0, :], xT_d[:, k, b * 512 : (b + 1) * 512]
                            )
                            xT_tiles[(k, b)] = t
                    b1_sb = c1p.tile([P, HO], f32)
                    nc.sync.dma_start(b1_sb[:], b1_d[:])

                    kxm1, kxm1_shape = dma_from_dram_kxm(kxm1p, w1_d[:])

                    # GEMM1 N axis: a full-512-tile batch plus a narrower tail
                    # batch so the 512-rounding of n1 is not paid in PE time.
                    # (keep the tail >= 256: f32r matmuls below 256 moving
                    # rows fall off the fast path)
                    nb0 = (n2 // 512) * 512
                    tail = n2 - nb0
                    if tail:
                        tail = max(tail, 256)
                    if nb0 == 0:
                        nb0, tail = tail, 0
                    fdims = (nb0, tail) if tail else (nb0,)
                    kxn1_shape = ShapeInfo(
                        pdims=((P, KO1),), fdims=fdims
                    )

                    def kxn1(nc_, md):
                        # K_TILE=128 => k_subtiles == 1; one xT shard per
                        # (k, 512-col block) so the first matmuls only gate on
                        # one small preload DMA, not the whole 18 MB of xT
                        start = md.n_batch_idx * nb0 + md.n_tile_idx * md.n_tile
                        blk, within = divmod(start, 512)
                        return xT_tiles[(md.k_tile_idx, blk)][
                            :, :, within : within + md.n_tile
                        ]

                    def hT_consumer(nc_, mxn_tile, md):
                        off = md.n_batch_idx * nb0
                        nss = md.n_slice_size
                        nc_.sync.dma_start(
                            hT_tiles[md.m_tile_idx][
                                :, :, ds(off + md.n_tile_idx * md.n_tile, nss)
                            ],
                            mxn_tile[:, :, :nss],
                        )

                    def gelu_reducer(nc_, psum, sbuf, md):
                        hc = md.m_tile_idx * md.m_subtiles + md.m_subtile_idx
                        nc_.scalar.activation(
                            sbuf[:, :, : md.n_slice_size],
                            psum[:, : md.n_slice_size],
                            mybir.ActivationFunctionType.Gelu,
                            bias=b1_sb[:, hc : hc + 1],
                        )

                    composable_matmul_tile_kernel(
                        tc=tc,
                        kxm_shape=kxm1_shape,
                        kxn_shape=kxn1_shape,
                        output_type=f16,
                        kxm_producer=kxm1,
                        kxn_producer=kxn1,
                        mxn_consumer=hT_consumer,
                        mxn_subtile_reducer=gelu_reducer,
                        MAX_K_TILE_SIZE=128,
                        skip_k_snake=True,
                        temps_n_bufs=3,
                        psum_n_bufs=2,
                    )

                # ---------------- phase 2: y = (hT.T @ w2 + b2) * scale -------------
                with tc.tile_pool(name="w2sb", bufs=1) as wp, \
                     tc.tile_pool(name="c2", bufs=1) as c2p, \
                     tc.tile_pool(name="kxm2", bufs=hidden // 512 + 1) as kxm2p:
                    K2_SUB = 4  # GEMM2 K_TILE=512 -> 4 ho chunks per k-tile
                    K2_TILES = HO // K2_SUB
                    w2_tiles = []
                    for j in range(K2_TILES):
                        t = wp.tile([P, K2_SUB, c_pad], f16, name=f"w2_{j}")
                        for s in range(K2_SUB):
                            nc.sync.dma_start(
                                t[:, s, :], w2_d[:, j * K2_SUB + s, :]
                            )
                        w2_tiles.append(t)
                    b2_sb = c2p.tile([P, c_pad], f32)
                    nc.sync.dma_start(b2_sb[:], b2_d[:])
                    sc_sb = c2p.tile([P, MO], f32)
                    nc.sync.dma_start(sc_sb[:], sc_d[:])

                    kxm2_shape = ShapeInfo(pdims=((P, HO),), fdims=(n2,))
                    kxn2_shape = ShapeInfo(pdims=((P, HO),), fdims=(c_pad,))

                    def kxm2(nc_, md):
                        # one hT shard per GEMM2 k-tile (K_TILE=512 aligns the
                        # shard boundary with GEMM1's m-tile output chunks)
                        t = kxm2p.tile(
                            [P, md.k_subtiles, md.m_tile], f16, tag="kxm2"
                        )
                        nc_.sync.dma_start(
                            t[:],
                            hT_tiles[md.k_tile_idx][
                                :, :, ds(md.m_tile_idx * md.m_tile, md.m_tile)
                            ],
                        )
                        return t

                    def kxn2(nc_, md):
                        return w2_tiles[md.k_tile_idx][
                            :, :, ds(md.n_tile_idx * md.n_tile, md.n_tile)
                        ]

                    def bias_scale_reducer(nc_, psum, sbuf, md):
                        nss = md.n_slice_size
                        cstart = md.n_tile_idx * md.n_tile + md.n_subtile_idx * md.n_subtile
                        tc_idx = md.m_tile_idx * md.m_subtiles + md.m_subtile_idx
                        nc_.vector.tensor_add(
                            out=sbuf[:, :, :nss],
                            in0=psum[:, :nss],
                            in1=b2_sb[:, cstart : cstart + nss],
                        )
                        nc_.scalar.activation(
                            sbuf[:, :, :nss],
                            sbuf[:, :, :nss],
                            mybir.ActivationFunctionType.Copy,
                            scale=sc_sb[:, tc_idx : tc_idx + 1],
                        )

                    composable_matmul_tile_kernel(
                        tc=tc,
                        kxm_shape=kxm2_shape,
                        kxn_shape=kxn2_shape,
                        output_type=f32,
                        kxm_producer=kxm2,
                        kxn_producer=kxn2,
                        mxn_consumer=dma_to_dram_mxn(y_d[:]),
                        mxn_subtile_reducer=bias_scale_reducer,
                        temps_n_bufs=3,
                        psum_n_bufs=2,
                    )


            for _rep in range(repeat):
                _emit_once()

    nc.compile()
    return nc


def _rearr_k(a, p=128):
    """[K, N] -> [p, K//p, N] with row k = ko*p + pi."""
    k, n = a.shape
    return np.ascontiguousarray(a.reshape(k // p, p, n).transpose(1, 0, 2))


def kernel(x, gate_w, gate_b, w1, b1, w2, b2):
    x = np.asarray(x, dtype=np.float32)
    gate_w = np.asarray(gate_w, dtype=np.float32)
    gate_b = np.asarray(gate_b, dtype=np.float32)
    w1 = np.asarray(w1, dtype=np.float32)
    b1 = np.asarray(b1, dtype=np.float32)
    w2 = np.asarray(w2, dtype=np.float32)
    b2 = np.asarray(b2, dtype=np.float32)

    B = x.shape[0]
    topk_idx, combine_w, aux = _gate(x, gate_w, gate_b)

    # dispatch: token lists + combine weights per expert
    idx_e = []
    scl_e = []
    for e in range(NUM_EXPERTS):
        sel = topk_idx == e  # [B, K] bool, at most one True per row
        tok = np.nonzero(sel.any(axis=1))[0]
        # boolean indexing walks row-major => weights come out in token order
        idx_e.append(tok)
        scl_e.append(combine_w[sel].astype(np.float32))

    n_max = max(len(t) for t in idx_e)
    n2 = max(256, int(math.ceil(n_max / 128.0)) * 128)
    n1 = int(math.ceil(n2 / 512.0)) * 512

    key = (n1, n2, D_MODEL, HIDDEN, C_PAD)
    if key not in _PROGRAM_CACHE:
        _PROGRAM_CACHE[key] = _build_program(n1, n2)
    nc = _PROGRAM_CACHE[key]

    P = 128
    MO = n2 // P
    in_maps = []
    for e in range(NUM_EXPERTS):
        tok = idx_e[e]
        xg = np.zeros((n1, D_MODEL), dtype=np.float32)
        xg[: len(tok)] = x[tok]
        sc = np.zeros((n2,), dtype=np.float32)
        sc[: len(tok)] = scl_e[e]
        w2p = np.zeros((HIDDEN, C_PAD), dtype=np.float16)
        w2p[:, :NUM_CLASSES] = w2[e].astype(np.float16)
        b2p = np.zeros((C_PAD,), dtype=np.float32)
        b2p[:NUM_CLASSES] = b2[e]
        in_maps.append(
            {
                "xT": _rearr_k(np.ascontiguousarray(xg.T)),
                "w1e": _rearr_k(w1[e]),
                "b1e": np.ascontiguousarray(b1[e].reshape(HIDDEN // P, P).T),
                "w2e": _rearr_k(w2p),
                "b2e": np.ascontiguousarray(
                    np.broadcast_to(b2p, (P, C_PAD))
                ),
                "scale": np.ascontiguousarray(sc.reshape(MO, P).T),
            }
        )

    from concourse.bass_utils import run_bass_kernel_spmd

    res = run_bass_kernel_spmd(nc, in_maps, list(range(NUM_EXPERTS)))

    _LAST_RUN["nc"] = nc
    _LAST_RUN["in_maps"] = in_maps
    _LAST_RUN["n1"] = n1
    _LAST_RUN["n2"] = n2

    y = np.zeros((B, NUM_CLASSES), dtype=np.float32)
    for e in range(NUM_EXPERTS):
        tok = idx_e[e]
        if len(tok) == 0:
            continue
        ye = res.results[e]["y"]  # [P, MO, C_PAD]
        ye = ye.transpose(1, 0, 2).reshape(n2, C_PAD)
        y[tok] += ye[: len(tok), :NUM_CLASSES]

    return y, aux



# revision 12
# speedup vs baseline: 1.0617x; 1.0617x over previous
"""MoE head (gate + top-2 expert MLPs + load-balance aux) on 8 Trainium2 cores.

Strategy: expert-parallel. The gate (0.05% of FLOPs) is computed on host with
jax ops mirroring the reference exactly (so top-2 selection bit-matches the
reference); tokens are dispatched to core e = expert e. Each core runs two
GEMMs on its ~4.1k routed tokens:

  phase 1:  hT[H, n] = w1[e].T @ x_gathered.T   (f32r matmuls, f32 PSUM)
            fused epilogue: gelu(psum + b1) -> fp16 hT in DRAM
  phase 2:  y[n, C]  = hT.T @ w2[e]             (fp16 matmuls, f32 PSUM)
            fused epilogue: (psum + b2) * combine_weight -> f32

Host then scatter-adds each expert's rows back into the full [B, 1000] output.
Rows the reference zero-weights contribute exactly 0 there, so routed compute
is mathematically identical to the dense reference loop.
"""

import math
import os
import sys

import numpy as np

for _p in ("/opt/trn_rl_repo", "/root/.axon_site/_ro/trn_rl_repo"):
    if os.path.isdir(_p) and _p not in sys.path:
        sys.path.insert(0, _p)

D_MODEL = 1024
NUM_CLASSES = 1000
NUM_EXPERTS = 8
TOP_K = 2
HIDDEN = D_MODEL * 4
TEMPERATURE = 1.0

# padded class dim (multiple of 512 keeps every tile full)
C_PAD = 1024

_PROGRAM_CACHE = {}
_LAST_RUN = {}


def _gate(x, gate_w, gate_b):
    """Gate math mirroring reference() exactly (same jax ops, same platform
    the harness runs the reference on) so that top-2 selection and aux agree
    to the bit whenever possible. Falls back to numpy if jax is unavailable."""
    try:
        import jax
        import jax.numpy as jnp

        xj = jnp.asarray(x)
        logits = (xj @ jnp.asarray(gate_w) + jnp.asarray(gate_b)) / TEMPERATURE
        probs = jax.nn.softmax(logits, axis=-1)
        topk_vals, topk_idx = jax.lax.top_k(probs, TOP_K)
        combine_w = topk_vals / jnp.sum(topk_vals, axis=-1, keepdims=True)
        onehot = jax.nn.one_hot(topk_idx, NUM_EXPERTS, dtype=xj.dtype)
        frac = jnp.mean(jnp.sum(onehot, axis=1), axis=0) / TOP_K
        imp = jnp.mean(probs, axis=0)
        aux = NUM_EXPERTS * jnp.sum(frac * imp)
        return (
            np.asarray(topk_idx),
            np.asarray(combine_w),
            np.asarray(aux, dtype=np.float32),
        )
    except Exception:
        logits = (x @ gate_w + gate_b) / TEMPERATURE
        m = logits.max(-1, keepdims=True)
        p = np.exp(logits - m, dtype=np.float32)
        probs = p / p.sum(-1, keepdims=True)
        topk_idx = np.argsort(-probs, axis=-1, kind="stable")[:, :TOP_K]
        topk_vals = np.take_along_axis(probs, topk_idx, axis=-1)
        combine_w = topk_vals / topk_vals.sum(-1, keepdims=True)
        counts = np.bincount(topk_idx.ravel(), minlength=NUM_EXPERTS)
        frac = counts.astype(np.float32) / (x.shape[0] * TOP_K)
        imp = probs.mean(0)
        aux = np.float32(NUM_EXPERTS * np.sum(frac * imp))
        return topk_idx, combine_w, aux


def _build_program(n1, n2=None, d_model=D_MODEL, hidden=HIDDEN, c_pad=C_PAD,
                   repeat=1):
    """One SPMD Bass program: per-core routed-expert MLP.

    n1: padded token count for GEMM1 (multiple of 512 — the cached-kxn path
        needs full 512-wide N tiles).
    n2: token count for GEMM2/output (multiple of 128, n2 <= n1) — GEMM2 only
        reads the first n2 columns of hT, so the 512-rounding waste of GEMM1
        is not paid again in GEMM2.
    repeat > 1 re-runs the whole compute that many times (same tensors) —
    used only for differential wall-clock timing, never for results."""
    if n2 is None:
        n2 = n1
    import concourse.mybir as mybir
    import concourse.tile as tile
    from concourse import bacc
    from concourse.bass import ds
    from concourse.kernels.tile_matmul import (
        ShapeInfo,
        composable_matmul_tile_kernel,
        dma_from_dram_kxm,
        dma_to_dram_mxn,
    )

    P = 128
    KO1 = d_model // P      # k-outer chunks for GEMM1
    HO = hidden // P        # h chunks of 128
    MO = n2 // P            # output token chunks of 128

    nc = bacc.Bacc("TRN2", target_bir_lowering=False, debug=False, num_devices=8)

    f32 = mybir.dt.float32
    f32r = mybir.dt.float32r
    f16 = mybir.dt.float16

    xT_d = nc.dram_tensor("xT", [P, KO1, n1], f32r, kind="ExternalInput")
    w1_d = nc.dram_tensor("w1e", [P, KO1, hidden], f32r, kind="ExternalInput")
    b1_d = nc.dram_tensor("b1e", [P, HO], f32, kind="ExternalInput")
    w2_d = nc.dram_tensor("w2e", [P, HO, c_pad], f16, kind="ExternalInput")
    b2_d = nc.dram_tensor("b2e", [P, c_pad], f32, kind="ExternalInput")
    sc_d = nc.dram_tensor("scale", [P, MO], f32, kind="ExternalInput")
    y_d = nc.dram_tensor("y", [P, MO, c_pad], f32, kind="ExternalOutput")

    HT_SPLIT = max(1, hidden // 512)  # one hT shard per GEMM1 m-tile
    with tile.TileContext(nc) as tc:
        with tc.tile_pool(name="dram", bufs=1, space="DRAM") as dramp:
            hT_tiles = [
                dramp.tile([P, HO // HT_SPLIT, n1], f16, name=f"hT{j}")
                for j in range(HT_SPLIT)
            ]

            def _emit_once():
                # ---------------- phase 1: hT = gelu(w1.T @ xT + b1) ----------------
                with tc.tile_pool(name="xsb", bufs=1) as xp, \
                     tc.tile_pool(name="c1", bufs=1) as c1p, \
                     tc.tile_pool(name="kxm1", bufs=d_model // P + 1) as kxm1p:
                    xT_tiles = {}

                    def _xT(k, b):
                        # demand-driven: the DMA is emitted at first use, so
                        # loads interleave with the matmul schedule instead of
                        # forming a 19 MB prologue the first matmul waits on
                        if (k, b) not in xT_tiles:
                            t = xp.tile([P, 1, 512], f32r, name=f"xT{k}_{b}")
                            nc.sync.dma_start(
                                t[:, 0, :], xT_d[:, k, b * 512 : (b + 1) * 512]
                            )
                            xT_tiles[(k, b)] = t
                        return xT_tiles[(k, b)]
                    b1_sb = c1p.tile([P, HO], f32)
                    nc.sync.dma_start(b1_sb[:], b1_d[:])

                    kxm1, kxm1_shape = dma_from_dram_kxm(kxm1p, w1_d[:])

                    # GEMM1 N axis: a full-512-tile batch plus a narrower tail
                    # batch so the 512-rounding of n1 is not paid in PE time.
                    # (keep the tail >= 256: f32r matmuls below 256 moving
                    # rows fall off the fast path)
                    nb0 = (n2 // 512) * 512
                    tail = n2 - nb0
                    if tail:
                        tail = max(tail, 256)
                    if nb0 == 0:
                        nb0, tail = tail, 0
                    fdims = (nb0, tail) if tail else (nb0,)
                    kxn1_shape = ShapeInfo(
                        pdims=((P, KO1),), fdims=fdims
                    )

                    def kxn1(nc_, md):
                        # K_TILE=128 => k_subtiles == 1; one xT shard per
                        # (k, 512-col block) so the first matmuls only gate on
                        # one small preload DMA, not the whole 18 MB of xT
                        start = md.n_batch_idx * nb0 + md.n_tile_idx * md.n_tile
                        blk, within = divmod(start, 512)
                        return _xT(md.k_tile_idx, blk)[
                            :, :, within : within + md.n_tile
                        ]

                    def hT_consumer(nc_, mxn_tile, md):
                        off = md.n_batch_idx * nb0
                        nss = md.n_slice_size
                        nc_.sync.dma_start(
                            hT_tiles[md.m_tile_idx][
                                :, :, ds(off + md.n_tile_idx * md.n_tile, nss)
                            ],
                            mxn_tile[:, :, :nss],
                        )

                    def gelu_reducer(nc_, psum, sbuf, md):
                        hc = md.m_tile_idx * md.m_subtiles + md.m_subtile_idx
                        nc_.scalar.activation(
                            sbuf[:, :, : md.n_slice_size],
                            psum[:, : md.n_slice_size],
                            mybir.ActivationFunctionType.Gelu,
                            bias=b1_sb[:, hc : hc + 1],
                        )

                    composable_matmul_tile_kernel(
                        tc=tc,
                        kxm_shape=kxm1_shape,
                        kxn_shape=kxn1_shape,
                        output_type=f16,
                        kxm_producer=kxm1,
                        kxn_producer=kxn1,
                        mxn_consumer=hT_consumer,
                        mxn_subtile_reducer=gelu_reducer,
                        MAX_K_TILE_SIZE=128,
                        skip_k_snake=True,
                        temps_n_bufs=3,
                        psum_n_bufs=2,
                    )

                # ---------------- phase 2: y = (hT.T @ w2 + b2) * scale -------------
                with tc.tile_pool(name="w2sb", bufs=1) as wp, \
                     tc.tile_pool(name="c2", bufs=1) as c2p, \
                     tc.tile_pool(name="kxm2", bufs=hidden // 512 + 1) as kxm2p:
                    K2_SUB = 4  # GEMM2 K_TILE=512 -> 4 ho chunks per k-tile
                    K2_TILES = HO // K2_SUB
                    w2_tiles = {}

                    def _w2(j):
                        # demand-driven for the same reason as _xT
                        if j not in w2_tiles:
                            t = wp.tile([P, K2_SUB, c_pad], f16, name=f"w2_{j}")
                            for s in range(K2_SUB):
                                nc.sync.dma_start(
                                    t[:, s, :], w2_d[:, j * K2_SUB + s, :]
                                )
                            w2_tiles[j] = t
                        return w2_tiles[j]
                    b2_sb = c2p.tile([P, c_pad], f32)
                    nc.sync.dma_start(b2_sb[:], b2_d[:])
                    sc_sb = c2p.tile([P, MO], f32)
                    nc.sync.dma_start(sc_sb[:], sc_d[:])

                    kxm2_shape = ShapeInfo(pdims=((P, HO),), fdims=(n2,))
                    kxn2_shape = ShapeInfo(pdims=((P, HO),), fdims=(c_pad,))

                    def kxm2(nc_, md):
                        # one hT shard per GEMM2 k-tile (K_TILE=512 aligns the
                        # shard boundary with GEMM1's m-tile output chunks)
                        t = kxm2p.tile(
                            [P, md.k_subtiles, md.m_tile], f16, tag="kxm2"
                        )
                        nc_.sync.dma_start(
                            t[:],
                            hT_tiles[md.k_tile_idx][
                                :, :, ds(md.m_tile_idx * md.m_tile, md.m_tile)
                            ],
                        )
                        return t

                    def kxn2(nc_, md):
                        return _w2(md.k_tile_idx)[
                            :, :, ds(md.n_tile_idx * md.n_tile, md.n_tile)
                        ]

                    def bias_scale_reducer(nc_, psum, sbuf, md):
                        nss = md.n_slice_size
                        cstart = md.n_tile_idx * md.n_tile + md.n_subtile_idx * md.n_subtile
                        tc_idx = md.m_tile_idx * md.m_subtiles + md.m_subtile_idx
                        nc_.vector.tensor_add(
                            out=sbuf[:, :, :nss],
                            in0=psum[:, :nss],
                            in1=b2_sb[:, cstart : cstart + nss],
                        )
                        nc_.scalar.activation(
                            sbuf[:, :, :nss],
                            sbuf[:, :, :nss],
                            mybir.ActivationFunctionType.Copy,
                            scale=sc_sb[:, tc_idx : tc_idx + 1],
                        )

                    composable_matmul_tile_kernel(
                        tc=tc,
                        kxm_shape=kxm2_shape,
                        kxn_shape=kxn2_shape,
                        output_type=f32,
                        kxm_producer=kxm2,
                        kxn_producer=kxn2,
                        mxn_consumer=dma_to_dram_mxn(y_d[:]),
                        mxn_subtile_reducer=bias_scale_reducer,
                        temps_n_bufs=3,
                        psum_n_bufs=2,
                    )


            for _rep in range(repeat):
                _emit_once()

    nc.compile()
    return nc


def _rearr_k(a, p=128):
    """[K, N] -> [p, K//p, N] with row k = ko*p + pi."""
    k, n = a.shape
    return np.ascontiguousarray(a.reshape(k // p, p, n).transpose(1, 0, 2))


def kernel(x, gate_w, gate_b, w1, b1, w2, b2):
    x = np.asarray(x, dtype=np.float32)
    gate_w = np.asarray(gate_w, dtype=np.float32)
    gate_b = np.asarray(gate_b, dtype=np.float32)
    w1 = np.asarray(w1, dtype=np.float32)
    b1 = np.asarray(b1, dtype=np.float32)
    w2 = np.asarray(w2, dtype=np.float32)
    b2 = np.asarray(b2, dtype=np.float32)

    B = x.shape[0]
    topk_idx, combine_w, aux = _gate(x, gate_w, gate_b)

    # dispatch: token lists + combine weights per expert
    idx_e = []
    scl_e = []
    for e in range(NUM_EXPERTS):
        sel = topk_idx == e  # [B, K] bool, at most one True per row
        tok = np.nonzero(sel.any(axis=1))[0]
        # boolean indexing walks row-major => weights come out in token order
        idx_e.append(tok)
        scl_e.append(combine_w[sel].astype(np.float32))

    n_max = max(len(t) for t in idx_e)
    n2 = max(256, int(math.ceil(n_max / 128.0)) * 128)
    n1 = int(math.ceil(n2 / 512.0)) * 512

    key = (n1, n2, D_MODEL, HIDDEN, C_PAD)
    if key not in _PROGRAM_CACHE:
        _PROGRAM_CACHE[key] = _build_program(n1, n2)
    nc = _PROGRAM_CACHE[key]

    P = 128
    MO = n2 // P
    in_maps = []
    for e in range(NUM_EXPERTS):
        tok = idx_e[e]
        xg = np.zeros((n1, D_MODEL), dtype=np.float32)
        xg[: len(tok)] = x[tok]
        sc = np.zeros((n2,), dtype=np.float32)
        sc[: len(tok)] = scl_e[e]
        w2p = np.zeros((HIDDEN, C_PAD), dtype=np.float16)
        w2p[:, :NUM_CLASSES] = w2[e].astype(np.float16)
        b2p = np.zeros((C_PAD,), dtype=np.float32)
        b2p[:NUM_CLASSES] = b2[e]
        in_maps.append(
            {
                "xT": _rearr_k(np.ascontiguousarray(xg.T)),
                "w1e": _rearr_k(w1[e]),
                "b1e": np.ascontiguousarray(b1[e].reshape(HIDDEN // P, P).T),
                "w2e": _rearr_k(w2p),
                "b2e": np.ascontiguousarray(
                    np.broadcast_to(b2p, (P, C_PAD))
                ),
                "scale": np.ascontiguousarray(sc.reshape(MO, P).T),
            }
        )

    from concourse.bass_utils import run_bass_kernel_spmd

    res = run_bass_kernel_spmd(nc, in_maps, list(range(NUM_EXPERTS)))

    _LAST_RUN["nc"] = nc
    _LAST_RUN["in_maps"] = in_maps
    _LAST_RUN["n1"] = n1
    _LAST_RUN["n2"] = n2

    y = np.zeros((B, NUM_CLASSES), dtype=np.float32)
    for e in range(NUM_EXPERTS):
        tok = idx_e[e]
        if len(tok) == 0:
            continue
        ye = res.results[e]["y"]  # [P, MO, C_PAD]
        ye = ye.transpose(1, 0, 2).reshape(n2, C_PAD)
        y[tok] += ye[: len(tok), :NUM_CLASSES]

    return y, aux

